# revision 11
# baseline (speedup 1.0000x reference)
"""Trainium2 Bass kernel for nn_DenseEntangler (B=256, D=32, L=3, 6 nodes).

Math: out = relu(bias + chain of 6 tensordot contractions). Each per-sample
contraction is a (1024 x 1024) matmul applied to the reshaped state, so the
whole problem is 6 matmuls of [1024,1024]^T @ [1024, Bc*32] per core
(Bc = 32 samples/core on 8 cores, batch-sharded).

Layout scheme (verified against the reference in numpy):
  state XT[(u*32+v) partition, (b*32+f) free], K = 1024 -> 8 tiles of 128.
  steps 0..4:  OUT[(n*32+m), (b,f)] = W_i^T @ XT  with
               W_i[(u*32+v), (n*32+m)] = nodes[i][u,v,m,n]  (host pre-permute)
               transition to the next step's XT = independent aligned 32x32
               block transposes (swap partition-low m with free-low f) ->
               native DVE stream_transpose, runs off the PE critical path.
  step 5:      operands swapped (state stationary, W5 moving) so PSUM comes
               out as [(b*32+f) partition, (m*32+n) free], which is
               DRAM-contiguous per partition for the final store.

Perf design (measured on HW):
  - PE roofline: 6 steps x 128 matmuls x 512 rows @ 2.4 GHz ~= 164 us; the
    achievable cadence is 227 ns per [128x512] matmul and steps run
    back-to-back at exactly that once fed.
  - matmul dtype is bfloat16 (same 1 cycle/row PE rate as float32r at
    N>=256, but HALF the DMA bytes). The head needs x + w0 resident before
    step 0's h0 pass completes; at the ~260 GB/s the fabric sustains early,
    fp32 head data (8 MiB) stalls the PE for ~20 us while bf16 (3 MiB in
    deadline order) arrives ahead of consumption. PSUM accumulation stays
    fp32; per-step round-to-bf16 of the state adds ~0.5% RMS error total
    (gate is 2e-2).
  - x is pre-transposed on the host into the XT k-tile layout so the input
    DMA is contiguous (2 KiB rows) instead of 128-byte strided gathers.
  - head DMAs are emitted in k-deadline order interleaved across the
    gpsimd/sync/scalar queues; payload DMA cannot start before ~9 us
    (engine boot), so the first matmul lands ~13 us in regardless.
  - a short self-serializing fp32 warm-up matmul burst ramps the PE clock
    out of its 1.2 GHz p-state during the DMA dead window.
"""

import os
import sys

import numpy as np

for _p in ("/opt/trn_rl_repo", "/root/.axon_site/_ro/trn_rl_repo"):
    if _p not in sys.path and os.path.isdir(_p):
        sys.path.append(_p)

B = 256
NCORES = 8
BC = B // NCORES  # 32 samples per core
NSTEP = 6
NK = 8  # K tiles of 128 (K = 1024)
NM = 8  # output partition tiles of 128 (steps 0..4)
NHALF = 2  # halves of 16 samples -> moving free dim 512
HB = BC // NHALF  # 16
NWARM = 2  # PE clock warm-up matmuls (fp32, 4 cyc/row -> ~1.7us each cold)

_NC_CACHE = {}


def _build_nc(mm_dtype_name):
    import concourse.tile as tile
    from concourse import bacc, mybir

    f32 = mybir.dt.float32
    mmdt = getattr(mybir.dt, mm_dtype_name)
    is_f32r = mmdt == mybir.dt.float32r
    # float32r has no numpy representation: declare those params as f32 and
    # bitcast the DMA source; true 16-bit dtypes are declared directly and
    # cast on the host.
    decl_dt = f32 if is_f32r else mmdt

    def src(ap):
        return ap.bitcast(mmdt) if is_f32r else ap

    # Bacc (not plain Bass): its lowering runs move_matmul_waits_to_ldweights
    # + generate_event_semaphores, required to satisfy the HW 1-wait-per-
    # instruction constraint on fused LDWEIGHTS+MATMUL.
    nc = bacc.Bacc(None)
    # x arrives pre-transposed from the host: x[k, pp, b*32+f] =
    # inputs[b, (k*128+pp)*32 + f] -> every DMA row is contiguous.
    xh = nc.declare_dram_parameter("x", [NK, 128, BC * 32], decl_dt, isOutput=False)
    wh = nc.declare_dram_parameter("w", [NSTEP, 128, 8192], decl_dt, isOutput=False)
    bh = nc.declare_dram_parameter("bias_in", [32768], f32, isOutput=False)
    yh = nc.declare_dram_parameter("y", [BC, 32768], f32, isOutput=True)

    # bias[(f*1024 + q)] -> [f, q]
    b2 = bh[:].rearrange("(f q) -> f q", q=1024)
    # y[b, f*1024 + q] -> [b, f, q]
    y3 = yh[:, :].rearrange("b (f q) -> b f q", q=1024)

    with tile.TileContext(nc) as tc:
        with (
            tc.tile_pool(name="wpool", bufs=16) as wpool,
            tc.tile_pool(name="xpool", bufs=32) as xpool,
            tc.tile_pool(name="wupool", bufs=1) as wupool,
            tc.tile_pool(name="bpool", bufs=1) as bpool,
            tc.tile_pool(name="tpool", bufs=4) as tpool,
            tc.tile_pool(name="stpool", bufs=4) as stpool,
            tc.tile_pool(name="opool", bufs=4) as opool,
            tc.tile_pool(name="pspool", bufs=8, space="PSUM") as pspool,
        ):
            wsb = {}

            def load_weights(step, eng_of=None):
                # steady-state weight stream: even k -> gpsimd (SWDGE,
                # ~half the bytes), odd k split between the two HWDGE
                # queues. Well under per-queue limits at bf16 rates.
                if eng_of is None:
                    eng_of = lambda k: (
                        nc.gpsimd
                        if k % 2 == 0
                        else (nc.sync if k % 4 == 1 else nc.scalar)
                    )
                tiles = []
                for k in range(NK):
                    t = wpool.tile([128, 1024], mmdt, tag="w")
                    eng_of(k).dma_start(
                        out=t[:],
                        in_=src(wh[step, :, k * 1024 : (k + 1) * 1024]),
                    )
                    tiles.append(t)
                wsb[step] = tiles

            # ---- PE clock warm-up: self-serializing dummy matmuls on a
            # zeroed tile into a scratch PSUM bank. No data dependencies, so
            # they issue as soon as the engines boot and ramp the PE out of
            # its 1.2 GHz p-state while the first x/w tiles are in flight.
            wu = wupool.tile([128, 512], f32, tag="wu")
            nc.vector.memset(wu[:], 0)
            wups = pspool.tile([128, 512], f32, tag="ps", name="warmps")
            for _ in range(NWARM):
                nc.tensor.matmul(
                    wups[:], wu[:, 0:128], wu[:], start=True, stop=True
                )

            # ---- head: (x[k], w0[k]) pairs land in k order. Whole 256 KiB
            # bf16 tiles over all three DGE queues: the urgent pairs k=0..4
            # alternate across sync/scalar (pair k at queue position k), the
            # relaxed tail pairs k=5..7 ride gpsimd, whose SWDGE spin-up
            # still beats their ~19-24 us deadlines. w1 follows in the gaps.
            x0 = [None] * NK
            wsb[0] = []
            for k in range(NK):
                x0[k] = xpool.tile(
                    [128, BC * 32], mmdt, tag="x0", name=f"x0_{k}", bufs=8
                )
                t = wpool.tile([128, 1024], mmdt, tag="w", name=f"w0_{k}")
                if k < 5:
                    xq, wq = (
                        (nc.sync, nc.scalar) if k % 2 == 0 else (nc.scalar, nc.sync)
                    )
                else:
                    xq = wq = nc.gpsimd
                xq.dma_start(out=x0[k][:], in_=src(xh[k]))
                wq.dma_start(
                    out=t[:], in_=src(wh[0, :, k * 1024 : (k + 1) * 1024])
                )
                wsb[0].append(t)

            load_weights(1)

            def finish_tile(ps, h, mt, xt_next):
                """PSUM -> (transpose, round-to-mmdt) -> next-step state tile."""
                if mmdt is f32:
                    t = xpool.tile([128, 512], f32, tag="xt")
                    nc.vector.transpose(t[:], ps[:])
                else:
                    st = stpool.tile([128, 512], f32, tag="st")
                    nc.vector.transpose(st[:], ps[:])
                    t = xpool.tile([128, 512], mmdt, tag="xt")
                    nc.scalar.copy(t[:], st[:])
                xt_next[h][mt] = t

            # ---- step 0: k-outer so PE consumes k-tiles in DMA arrival order
            xt_next = [[None] * NK for _ in range(NHALF)]
            for h in range(NHALF):
                pss = [
                    pspool.tile([128, 512], f32, tag="ps", name=f"ps0_{h}_{i}")
                    for i in range(NM)
                ]
                for k in range(NK):
                    for mt in range(NM):
                        nc.tensor.matmul(
                            pss[mt][:],
                            wsb[0][k][:, mt * 128 : (mt + 1) * 128],
                            x0[k][:, h * 512 : (h + 1) * 512],
                            start=(k == 0),
                            stop=(k == NK - 1),
                        )
                for mt in range(NM):
                    finish_tile(pss[mt], h, mt, xt_next)
            load_weights(2)
            xt = xt_next

            # ---- steps 1..4: mt-outer (staggers transposes across the step)
            for step in range(1, 5):
                xt_next = [[None] * NK for _ in range(NHALF)]
                for h in range(NHALF):
                    for mt in range(NM):
                        ps = pspool.tile([128, 512], f32, tag="ps")
                        for k in range(NK):
                            nc.tensor.matmul(
                                ps[:],
                                wsb[step][k][:, mt * 128 : (mt + 1) * 128],
                                xt[h][k][:],
                                start=(k == 0),
                                stop=(k == NK - 1),
                            )
                        finish_tile(ps, h, mt, xt_next)
                if step + 2 < NSTEP:
                    load_weights(step + 2)
                xt = xt_next

            # ---- step 5: state stationary, W moving; fused bias+relu+store ----
            from concourse.mybir import ActivationFunctionType

            # bias tile: [128, 1024], row p holds bias[(p%32)*1024 : ...];
            # loaded late, right before its only consumer.
            bias_sb = bpool.tile([128, 1024], f32, tag="bias")
            for r in range(4):
                nc.sync.dma_start(out=bias_sb[32 * r : 32 * (r + 1), :], in_=b2[:, :])

            for h in range(NHALF):
                for mc in range(4):  # output partition chunks of 128 (= 4 b values)
                    for nh in range(2):  # N halves of 512
                        ps = pspool.tile([128, 512], f32, tag="ps")
                        for k in range(NK):
                            nc.tensor.matmul(
                                ps[:],
                                xt[h][k][:, mc * 128 : (mc + 1) * 128],
                                wsb[5][k][:, nh * 512 : (nh + 1) * 512],
                                start=(k == 0),
                                stop=(k == NK - 1),
                            )
                        tmp = tpool.tile([128, 512], f32, tag="tmp")
                        nc.vector.tensor_add(
                            tmp[:], ps[:], bias_sb[:, nh * 512 : (nh + 1) * 512]
                        )
                        o = opool.tile([128, 512], f32, tag="o")
                        nc.scalar.activation(o[:], tmp[:], ActivationFunctionType.Relu)
                        b0 = h * HB + mc * 4
                        if h == 1 and mc == 3 and nh == 1:
                            # the very last chunk: split the store across two
                            # queues so the drain tail halves.
                            nc.sync.dma_start(
                                out=y3[b0 : b0 + 2, :, nh * 512 : (nh + 1) * 512],
                                in_=o[0:64, :],
                            )
                            nc.scalar.dma_start(
                                out=y3[b0 + 2 : b0 + 4, :, nh * 512 : (nh + 1) * 512],
                                in_=o[64:128, :],
                            )
                        else:
                            nc.sync.dma_start(
                                out=y3[b0 : b0 + 4, :, nh * 512 : (nh + 1) * 512],
                                in_=o[:],
                            )
    # Run the Bacc lowering passes (register allocation, wait splitting, ...)
    # — the PJRT execute path serializes nc.m as-is.
    nc.finalize()
    return nc


def _get_nc(mm_dtype_name):
    if mm_dtype_name not in _NC_CACHE:
        _NC_CACHE[mm_dtype_name] = _build_nc(mm_dtype_name)
    return _NC_CACHE[mm_dtype_name]


def _host_dt(mm_dtype_name):
    if mm_dtype_name == "float32r":
        return np.float32
    import ml_dtypes

    return np.dtype(getattr(ml_dtypes, mm_dtype_name))


def _prep_weights(nodes, host_dt):
    # W[i] layout [p=(u*32+v)%... rows 128 per k-tile packed as [128, 8*1024]]:
    # free index = k*1024 + col.  steps 0..4: col = n*32+m ; step 5: col = m*32+n.
    nodes = np.ascontiguousarray(nodes, dtype=np.float32)
    W = np.empty((NSTEP, 128, 8192), np.float32)
    for i in range(NSTEP):
        if i < 5:
            wm = nodes[i].reshape(1024, 32, 32).transpose(0, 2, 1).reshape(1024, 1024)
        else:
            wm = nodes[i].reshape(1024, 1024)
        # [k*128+p, col] -> [p, k*1024+col]
        W[i] = wm.reshape(NK, 128, 1024).transpose(1, 0, 2).reshape(128, 8192)
    return np.ascontiguousarray(W.astype(host_dt))


def _prep_x(x_core, host_dt):
    # [BC, 32768] -> [k, pp, b*32+f] with x[b, (k*128+pp)*32+f]
    return np.ascontiguousarray(
        x_core.reshape(BC, NK, 128, 32)
        .transpose(1, 2, 0, 3)
        .reshape(NK, 128, BC * 32)
        .astype(host_dt)
    )


def run(inputs, nodes, bias, mm_dtype="bfloat16", trace=False):
    from concourse.bass_utils import run_bass_kernel_spmd

    nc = _get_nc(mm_dtype)
    hdt = _host_dt(mm_dtype)
    x = np.ascontiguousarray(inputs, dtype=np.float32)
    bias = np.ascontiguousarray(bias, dtype=np.float32)
    W = _prep_weights(nodes, hdt)
    in_maps = [
        {"x": _prep_x(x[c * BC : (c + 1) * BC], hdt), "w": W, "bias_in": bias}
        for c in range(NCORES)
    ]
    res = run_bass_kernel_spmd(nc, in_maps, list(range(NCORES)), trace=trace)
    out = np.concatenate([res.results[c]["y"] for c in range(NCORES)], axis=0)
    return out, res


def kernel(inputs, nodes, bias):
    out, _ = run(inputs, nodes, bias)
    return out


# revision 15
# speedup vs baseline: 1.0157x; 1.0157x over previous
"""Trainium2 Bass kernel for nn_DenseEntangler (B=256, D=32, L=3, 6 nodes).

Math: out = relu(bias + chain of 6 tensordot contractions). Each per-sample
contraction is a (1024 x 1024) matmul applied to the reshaped state, so the
whole problem is 6 matmuls of [1024,1024]^T @ [1024, Bc*32] per core
(Bc = 32 samples/core on 8 cores, batch-sharded).

Layout scheme (verified against the reference in numpy):
  state XT[(u*32+v) partition, (b*32+f) free], K = 1024 -> 8 tiles of 128.
  steps 0..4:  OUT[(n*32+m), (b,f)] = W_i^T @ XT  with
               W_i[(u*32+v), (n*32+m)] = nodes[i][u,v,m,n]  (host pre-permute)
               transition to the next step's XT = independent aligned 32x32
               block transposes (swap partition-low m with free-low f) ->
               native DVE stream_transpose, runs off the PE critical path.
  step 5:      operands swapped (state stationary, W5 moving) so PSUM comes
               out as [(b*32+f) partition, (m*32+n) free], which is
               DRAM-contiguous per partition for the final store.

Perf design (measured on HW):
  - PE roofline: 6 steps x 128 matmuls x 512 rows @ 2.4 GHz ~= 164 us; the
    achievable cadence is 227 ns per [128x512] matmul and steps run
    back-to-back at exactly that once fed.
  - matmul dtype is bfloat16 (same 1 cycle/row PE rate as float32r at
    N>=256, but HALF the DMA bytes). The head needs x + w0 resident before
    step 0's h0 pass completes; at the ~260 GB/s the fabric sustains early,
    fp32 head data (8 MiB) stalls the PE for ~20 us while bf16 (3 MiB in
    deadline order) arrives ahead of consumption. PSUM accumulation stays
    fp32; per-step round-to-bf16 of the state adds ~0.5% RMS error total
    (gate is 2e-2).
  - x is pre-transposed on the host into the XT k-tile layout so the input
    DMA is contiguous (2 KiB rows) instead of 128-byte strided gathers.
  - head DMAs are emitted in k-deadline order interleaved across the
    gpsimd/sync/scalar queues; payload DMA cannot start before ~9 us
    (engine boot), so the first matmul lands ~13 us in regardless.
  - a short self-serializing fp32 warm-up matmul burst ramps the PE clock
    out of its 1.2 GHz p-state during the DMA dead window.
"""

import os
import sys

import numpy as np

for _p in ("/opt/trn_rl_repo", "/root/.axon_site/_ro/trn_rl_repo"):
    if _p not in sys.path and os.path.isdir(_p):
        sys.path.append(_p)

B = 256
NCORES = 8
BC = B // NCORES  # 32 samples per core
NSTEP = 6
NK = 8  # K tiles of 128 (K = 1024)
NM = 8  # output partition tiles of 128 (steps 0..4)
NHALF = 2  # halves of 16 samples -> moving free dim 512
HB = BC // NHALF  # 16
NWARM = 2  # PE clock warm-up matmuls (fp32, 4 cyc/row -> ~1.7us each cold)

_NC_CACHE = {}


def _build_nc(mm_dtype_name):
    import concourse.tile as tile
    from concourse import bacc, mybir

    f32 = mybir.dt.float32
    mmdt = getattr(mybir.dt, mm_dtype_name)
    is_f32r = mmdt == mybir.dt.float32r
    # float32r has no numpy representation: declare those params as f32 and
    # bitcast the DMA source; true 16-bit dtypes are declared directly and
    # cast on the host.
    decl_dt = f32 if is_f32r else mmdt

    def src(ap):
        return ap.bitcast(mmdt) if is_f32r else ap

    # Bacc (not plain Bass): its lowering runs move_matmul_waits_to_ldweights
    # + generate_event_semaphores, required to satisfy the HW 1-wait-per-
    # instruction constraint on fused LDWEIGHTS+MATMUL.
    nc = bacc.Bacc(None)
    # x arrives pre-transposed from the host: x[k, pp, b*32+f] =
    # inputs[b, (k*128+pp)*32 + f] -> every DMA row is contiguous.
    xh = nc.declare_dram_parameter("x", [NK, 128, BC * 32], decl_dt, isOutput=False)
    wh = nc.declare_dram_parameter("w", [NSTEP, 128, 8192], decl_dt, isOutput=False)
    bh = nc.declare_dram_parameter("bias_in", [32768], f32, isOutput=False)
    yh = nc.declare_dram_parameter("y", [BC, 32768], f32, isOutput=True)

    # bias[(f*1024 + q)] -> [f, q]
    b2 = bh[:].rearrange("(f q) -> f q", q=1024)
    # y[b, f*1024 + q] -> [b, f, q]
    y3 = yh[:, :].rearrange("b (f q) -> b f q", q=1024)

    with tile.TileContext(nc) as tc:
        with (
            tc.tile_pool(name="wpool", bufs=16) as wpool,
            tc.tile_pool(name="xpool", bufs=32) as xpool,
            tc.tile_pool(name="wupool", bufs=1) as wupool,
            tc.tile_pool(name="bpool", bufs=1) as bpool,
            tc.tile_pool(name="tpool", bufs=4) as tpool,
            tc.tile_pool(name="stpool", bufs=4) as stpool,
            tc.tile_pool(name="opool", bufs=4) as opool,
            tc.tile_pool(name="pspool", bufs=8, space="PSUM") as pspool,
        ):
            wsb = {}

            def load_weights(step, eng_of=None):
                # steady-state weight stream: even k -> gpsimd (SWDGE,
                # ~half the bytes), odd k split between the two HWDGE
                # queues. Well under per-queue limits at bf16 rates.
                if eng_of is None:
                    eng_of = lambda k: (
                        nc.gpsimd
                        if k % 2 == 0
                        else (nc.sync if k % 4 == 1 else nc.scalar)
                    )
                tiles = []
                for k in range(NK):
                    t = wpool.tile([128, 1024], mmdt, tag="w")
                    eng_of(k).dma_start(
                        out=t[:],
                        in_=src(wh[step, :, k * 1024 : (k + 1) * 1024]),
                    )
                    tiles.append(t)
                wsb[step] = tiles

            # ---- PE clock warm-up: self-serializing dummy matmuls on a
            # zeroed tile into a scratch PSUM bank. No data dependencies, so
            # they issue as soon as the engines boot and ramp the PE out of
            # its 1.2 GHz p-state while the first x/w tiles are in flight.
            wu = wupool.tile([128, 512], f32, tag="wu")
            nc.vector.memset(wu[:], 0)
            wups = pspool.tile([128, 512], f32, tag="ps", name="warmps")
            for _ in range(NWARM):
                nc.tensor.matmul(
                    wups[:], wu[:, 0:128], wu[:], start=True, stop=True
                )

            # ---- head. Measured per-queue caps: sync/scalar ~78 GB/s each,
            # gpsimd ~112 GB/s (byte-bound, not packet-bound), vs a step-0
            # h0-pass demand of 384 KiB per 1.73 us k-slot (222 GB/s). So
            # every k-slot's bytes are split across all three queues in
            # proportion to their rates, in strict k order:
            #   gpsimd: x h0-half (128K) + w0 cols 0:128   (32K)  -> 92 GB/s
            #   sync:   w0 cols 128:576                    (112K) -> 65 GB/s
            #   scalar: w0 cols 576:1024                   (112K) -> 65 GB/s
            # The x h1-halves follow on gpsimd (needed only by the h1 pass),
            # then w1 split sync/scalar, all ahead of their deadlines.
            x0 = [None] * NK
            wsb[0] = []
            for k in range(NK):
                x0[k] = xpool.tile(
                    [128, BC * 32], mmdt, tag="x0", name=f"x0_{k}", bufs=8
                )
                t = wpool.tile([128, 1024], mmdt, tag="w", name=f"w0_{k}")
                c0 = k * 1024
                nc.gpsimd.dma_start(
                    out=x0[k][:, 0 : HB * 32], in_=src(xh[k, :, 0 : HB * 32])
                )
                nc.gpsimd.dma_start(
                    out=t[:, 0:128], in_=src(wh[0, :, c0 : c0 + 128])
                )
                nc.sync.dma_start(
                    out=t[:, 128:576], in_=src(wh[0, :, c0 + 128 : c0 + 576])
                )
                nc.scalar.dma_start(
                    out=t[:, 576:1024], in_=src(wh[0, :, c0 + 576 : c0 + 1024])
                )
                wsb[0].append(t)
            for k in range(NK):
                nc.gpsimd.dma_start(
                    out=x0[k][:, HB * 32 : BC * 32],
                    in_=src(xh[k, :, HB * 32 : BC * 32]),
                )

            # w1: halves on the two HWDGE queues, done ~5 us before step 1.
            def load_weights_split(step):
                tiles = []
                for k in range(NK):
                    t = wpool.tile([128, 1024], mmdt, tag="w")
                    nc.sync.dma_start(
                        out=t[:, 0:512], in_=src(wh[step, :, k * 1024 : k * 1024 + 512])
                    )
                    nc.scalar.dma_start(
                        out=t[:, 512:1024],
                        in_=src(wh[step, :, k * 1024 + 512 : (k + 1) * 1024]),
                    )
                    tiles.append(t)
                wsb[step] = tiles

            load_weights_split(1)

            def finish_tile(ps, h, mt, xt_next):
                """PSUM -> (transpose, round-to-mmdt) -> next-step state tile."""
                if mmdt is f32:
                    t = xpool.tile([128, 512], f32, tag="xt")
                    nc.vector.transpose(t[:], ps[:])
                else:
                    st = stpool.tile([128, 512], f32, tag="st")
                    nc.vector.transpose(st[:], ps[:])
                    t = xpool.tile([128, 512], mmdt, tag="xt")
                    nc.scalar.copy(t[:], st[:])
                xt_next[h][mt] = t

            # ---- step 0: k-outer so PE consumes k-tiles in DMA arrival
            # order; mt rotated so mt=0 (whose w0 column block rides the
            # slightly-later gpsimd piece) is consumed last within each k.
            xt_next = [[None] * NK for _ in range(NHALF)]
            for h in range(NHALF):
                pss = [
                    pspool.tile([128, 512], f32, tag="ps", name=f"ps0_{h}_{i}")
                    for i in range(NM)
                ]
                for k in range(NK):
                    for mt in (1, 2, 3, 4, 5, 6, 7, 0):
                        nc.tensor.matmul(
                            pss[mt][:],
                            wsb[0][k][:, mt * 128 : (mt + 1) * 128],
                            x0[k][:, h * 512 : (h + 1) * 512],
                            start=(k == 0),
                            stop=(k == NK - 1),
                        )
                for mt in range(NM):
                    finish_tile(pss[mt], h, mt, xt_next)
            load_weights(2)
            xt = xt_next

            # ---- steps 1..4: mt-outer (staggers transposes across the step)
            for step in range(1, 5):
                xt_next = [[None] * NK for _ in range(NHALF)]
                for h in range(NHALF):
                    for mt in range(NM):
                        ps = pspool.tile([128, 512], f32, tag="ps")
                        for k in range(NK):
                            nc.tensor.matmul(
                                ps[:],
                                wsb[step][k][:, mt * 128 : (mt + 1) * 128],
                                xt[h][k][:],
                                start=(k == 0),
                                stop=(k == NK - 1),
                            )
                        finish_tile(ps, h, mt, xt_next)
                if step + 2 < NSTEP:
                    load_weights(step + 2)
                xt = xt_next

            # ---- step 5: state stationary, W moving; fused bias+relu+store ----
            from concourse.mybir import ActivationFunctionType

            # bias tile: [128, 1024], row p holds bias[(p%32)*1024 : ...];
            # loaded late, right before its only consumer.
            bias_sb = bpool.tile([128, 1024], f32, tag="bias")
            for r in range(4):
                nc.sync.dma_start(out=bias_sb[32 * r : 32 * (r + 1), :], in_=b2[:, :])

            for h in range(NHALF):
                for mc in range(4):  # output partition chunks of 128 (= 4 b values)
                    for nh in range(2):  # N halves of 512
                        ps = pspool.tile([128, 512], f32, tag="ps")
                        for k in range(NK):
                            nc.tensor.matmul(
                                ps[:],
                                xt[h][k][:, mc * 128 : (mc + 1) * 128],
                                wsb[5][k][:, nh * 512 : (nh + 1) * 512],
                                start=(k == 0),
                                stop=(k == NK - 1),
                            )
                        tmp = tpool.tile([128, 512], f32, tag="tmp")
                        nc.vector.tensor_add(
                            tmp[:], ps[:], bias_sb[:, nh * 512 : (nh + 1) * 512]
                        )
                        o = opool.tile([128, 512], f32, tag="o")
                        nc.scalar.activation(o[:], tmp[:], ActivationFunctionType.Relu)
                        b0 = h * HB + mc * 4
                        nc.sync.dma_start(
                            out=y3[b0 : b0 + 4, :, nh * 512 : (nh + 1) * 512],
                            in_=o[:],
                        )
    # Run the Bacc lowering passes (register allocation, wait splitting, ...)
    # — the PJRT execute path serializes nc.m as-is.
    nc.finalize()
    return nc


def _get_nc(mm_dtype_name):
    if mm_dtype_name not in _NC_CACHE:
        _NC_CACHE[mm_dtype_name] = _build_nc(mm_dtype_name)
    return _NC_CACHE[mm_dtype_name]


def _host_dt(mm_dtype_name):
    if mm_dtype_name == "float32r":
        return np.float32
    import ml_dtypes

    return np.dtype(getattr(ml_dtypes, mm_dtype_name))


def _prep_weights(nodes, host_dt):
    # W[i] layout [p=(u*32+v)%... rows 128 per k-tile packed as [128, 8*1024]]:
    # free index = k*1024 + col.  steps 0..4: col = n*32+m ; step 5: col = m*32+n.
    nodes = np.ascontiguousarray(nodes, dtype=np.float32)
    W = np.empty((NSTEP, 128, 8192), np.float32)
    for i in range(NSTEP):
        if i < 5:
            wm = nodes[i].reshape(1024, 32, 32).transpose(0, 2, 1).reshape(1024, 1024)
        else:
            wm = nodes[i].reshape(1024, 1024)
        # [k*128+p, col] -> [p, k*1024+col]
        W[i] = wm.reshape(NK, 128, 1024).transpose(1, 0, 2).reshape(128, 8192)
    return np.ascontiguousarray(W.astype(host_dt))


def _prep_x(x_core, host_dt):
    # [BC, 32768] -> [k, pp, b*32+f] with x[b, (k*128+pp)*32+f]
    return np.ascontiguousarray(
        x_core.reshape(BC, NK, 128, 32)
        .transpose(1, 2, 0, 3)
        .reshape(NK, 128, BC * 32)
        .astype(host_dt)
    )


def run(inputs, nodes, bias, mm_dtype="bfloat16", trace=False):
    from concourse.bass_utils import run_bass_kernel_spmd

    nc = _get_nc(mm_dtype)
    hdt = _host_dt(mm_dtype)
    x = np.ascontiguousarray(inputs, dtype=np.float32)
    bias = np.ascontiguousarray(bias, dtype=np.float32)
    W = _prep_weights(nodes, hdt)
    in_maps = [
        {"x": _prep_x(x[c * BC : (c + 1) * BC], hdt), "w": W, "bias_in": bias}
        for c in range(NCORES)
    ]
    res = run_bass_kernel_spmd(nc, in_maps, list(range(NCORES)), trace=trace)
    out = np.concatenate([res.results[c]["y"] for c in range(NCORES)], axis=0)
    return out, res


def kernel(inputs, nodes, bias):
    out, _ = run(inputs, nodes, bias)
    return out


# revision 18
# speedup vs baseline: 1.0224x; 1.0066x over previous
"""Trainium2 Bass kernel for nn_DenseEntangler (B=256, D=32, L=3, 6 nodes).

Math: out = relu(bias + chain of 6 tensordot contractions). Each per-sample
contraction is a (1024 x 1024) matmul applied to the reshaped state, so the
whole problem is 6 matmuls of [1024,1024]^T @ [1024, Bc*32] per core
(Bc = 32 samples/core on 8 cores, batch-sharded).

Layout scheme (verified against the reference in numpy):
  state XT[(u*32+v) partition, (b*32+f) free], K = 1024 -> 8 tiles of 128.
  steps 0..4:  OUT[(n*32+m), (b,f)] = W_i^T @ XT  with
               W_i[(u*32+v), (n*32+m)] = nodes[i][u,v,m,n]  (host pre-permute)
               transition to the next step's XT = independent aligned 32x32
               block transposes (swap partition-low m with free-low f) ->
               native DVE stream_transpose, runs off the PE critical path.
  step 5:      operands swapped (state stationary, W5 moving) so PSUM comes
               out as [(b*32+f) partition, (m*32+n) free], which is
               DRAM-contiguous per partition for the final store.

Perf design (measured on HW):
  - PE roofline: 6 steps x 128 matmuls x 512 rows @ 2.4 GHz ~= 164 us; the
    achievable cadence is 227 ns per [128x512] matmul and steps run
    back-to-back at exactly that once fed.
  - matmul dtype is bfloat16 (same 1 cycle/row PE rate as float32r at
    N>=256, but HALF the DMA bytes). The head needs x + w0 resident before
    step 0's h0 pass completes; at the ~260 GB/s the fabric sustains early,
    fp32 head data (8 MiB) stalls the PE for ~20 us while bf16 (3 MiB in
    deadline order) arrives ahead of consumption. PSUM accumulation stays
    fp32; per-step round-to-bf16 of the state adds ~0.5% RMS error total
    (gate is 2e-2).
  - x is pre-transposed on the host into the XT k-tile layout so the input
    DMA is contiguous (2 KiB rows) instead of 128-byte strided gathers.
  - head DMAs are emitted in k-deadline order interleaved across the
    gpsimd/sync/scalar queues; payload DMA cannot start before ~9 us
    (engine boot), so the first matmul lands ~13 us in regardless.
  - a short self-serializing fp32 warm-up matmul burst ramps the PE clock
    out of its 1.2 GHz p-state during the DMA dead window.
"""

import os
import sys

import numpy as np

for _p in ("/opt/trn_rl_repo", "/root/.axon_site/_ro/trn_rl_repo"):
    if _p not in sys.path and os.path.isdir(_p):
        sys.path.append(_p)

B = 256
NCORES = 8
BC = B // NCORES  # 32 samples per core
NSTEP = 6
NK = 8  # K tiles of 128 (K = 1024)
NM = 8  # output partition tiles of 128 (steps 0..4)
NHALF = 2  # halves of 16 samples -> moving free dim 512
HB = BC // NHALF  # 16
NWARM = 2  # PE clock warm-up matmuls (fp32, 4 cyc/row -> ~1.7us each cold)

_NC_CACHE = {}


def _build_nc(mm_dtype_name):
    import concourse.tile as tile
    from concourse import bacc, mybir

    f32 = mybir.dt.float32
    mmdt = getattr(mybir.dt, mm_dtype_name)
    is_f32r = mmdt == mybir.dt.float32r
    # float32r has no numpy representation: declare those params as f32 and
    # bitcast the DMA source; true 16-bit dtypes are declared directly and
    # cast on the host.
    decl_dt = f32 if is_f32r else mmdt

    def src(ap):
        return ap.bitcast(mmdt) if is_f32r else ap

    # Bacc (not plain Bass): its lowering runs move_matmul_waits_to_ldweights
    # + generate_event_semaphores, required to satisfy the HW 1-wait-per-
    # instruction constraint on fused LDWEIGHTS+MATMUL.
    nc = bacc.Bacc(None)
    # x arrives pre-transposed from the host: x[k, pp, b*32+f] =
    # inputs[b, (k*128+pp)*32 + f] -> every DMA row is contiguous.
    xh = nc.declare_dram_parameter("x", [NK, 128, BC * 32], decl_dt, isOutput=False)
    wh = nc.declare_dram_parameter("w", [NSTEP, 128, 8192], decl_dt, isOutput=False)
    bh = nc.declare_dram_parameter("bias_in", [32768], f32, isOutput=False)
    yh = nc.declare_dram_parameter("y", [BC, 32768], f32, isOutput=True)

    # bias[(f*1024 + q)] -> [f, q]
    b2 = bh[:].rearrange("(f q) -> f q", q=1024)
    # y[b, f*1024 + q] -> [b, f, q]
    y3 = yh[:, :].rearrange("b (f q) -> b f q", q=1024)

    with tile.TileContext(nc) as tc:
        with (
            tc.tile_pool(name="wpool", bufs=16) as wpool,
            tc.tile_pool(name="xpool", bufs=32) as xpool,
            tc.tile_pool(name="wupool", bufs=1) as wupool,
            tc.tile_pool(name="bpool", bufs=1) as bpool,
            tc.tile_pool(name="tpool", bufs=4) as tpool,
            tc.tile_pool(name="stpool", bufs=4) as stpool,
            tc.tile_pool(name="opool", bufs=4) as opool,
            tc.tile_pool(name="pspool", bufs=8, space="PSUM") as pspool,
        ):
            wsb = {}

            def load_weights(step, eng_of=None):
                # steady-state weight stream: even k -> gpsimd (SWDGE,
                # ~half the bytes), odd k split between the two HWDGE
                # queues. Well under per-queue limits at bf16 rates.
                if eng_of is None:
                    eng_of = lambda k: (
                        nc.gpsimd
                        if k % 2 == 0
                        else (nc.sync if k % 4 == 1 else nc.scalar)
                    )
                tiles = []
                for k in range(NK):
                    t = wpool.tile([128, 1024], mmdt, tag="w")
                    eng_of(k).dma_start(
                        out=t[:],
                        in_=src(wh[step, :, k * 1024 : (k + 1) * 1024]),
                    )
                    tiles.append(t)
                wsb[step] = tiles

            # ---- PE clock warm-up: self-serializing dummy matmuls on a
            # zeroed tile into a scratch PSUM bank. No data dependencies, so
            # they issue as soon as the engines boot and ramp the PE out of
            # its 1.2 GHz p-state while the first x/w tiles are in flight.
            wu = wupool.tile([128, 512], f32, tag="wu")
            nc.vector.memset(wu[:], 0)
            wups = pspool.tile([128, 512], f32, tag="ps", name="warmps")
            for _ in range(NWARM):
                nc.tensor.matmul(
                    wups[:], wu[:, 0:128], wu[:], start=True, stop=True
                )

            # ---- head. Measured per-queue caps: sync/scalar ~78-85 GB/s
            # each, gpsimd ~95-112 GB/s (byte-bound, not packet-bound), vs a
            # step-0 h0-pass demand of 384 KiB per 1.73 us k-slot (222 GB/s).
            # Every k-slot's bytes are split in equal thirds across the three
            # queues, in strict k order:
            #   gpsimd: x h0-half      (128K)
            #   sync:   w0 cols 0:512  (128K)
            #   scalar: w0 cols 512:.. (128K)
            # The x h1-halves follow on gpsimd (needed only by the h1 pass),
            # then w1 split sync/scalar, all ahead of their deadlines.
            x0 = [None] * NK
            wsb[0] = []
            for k in range(NK):
                x0[k] = xpool.tile(
                    [128, BC * 32], mmdt, tag="x0", name=f"x0_{k}", bufs=8
                )
                t = wpool.tile([128, 1024], mmdt, tag="w", name=f"w0_{k}")
                c0 = k * 1024
                nc.gpsimd.dma_start(
                    out=x0[k][:, 0 : HB * 32], in_=src(xh[k, :, 0 : HB * 32])
                )
                nc.sync.dma_start(
                    out=t[:, 0:512], in_=src(wh[0, :, c0 : c0 + 512])
                )
                nc.scalar.dma_start(
                    out=t[:, 512:1024], in_=src(wh[0, :, c0 + 512 : c0 + 1024])
                )
                wsb[0].append(t)
            for k in range(NK):
                nc.gpsimd.dma_start(
                    out=x0[k][:, HB * 32 : BC * 32],
                    in_=src(xh[k, :, HB * 32 : BC * 32]),
                )

            # w1: halves on the two HWDGE queues, done ~5 us before step 1.
            def load_weights_split(step):
                tiles = []
                for k in range(NK):
                    t = wpool.tile([128, 1024], mmdt, tag="w")
                    nc.sync.dma_start(
                        out=t[:, 0:512], in_=src(wh[step, :, k * 1024 : k * 1024 + 512])
                    )
                    nc.scalar.dma_start(
                        out=t[:, 512:1024],
                        in_=src(wh[step, :, k * 1024 + 512 : (k + 1) * 1024]),
                    )
                    tiles.append(t)
                wsb[step] = tiles

            load_weights_split(1)

            def finish_tile(ps, h, mt, xt_next):
                """PSUM -> (transpose, round-to-mmdt) -> next-step state tile."""
                if mmdt is f32:
                    t = xpool.tile([128, 512], f32, tag="xt")
                    nc.vector.transpose(t[:], ps[:])
                else:
                    st = stpool.tile([128, 512], f32, tag="st")
                    nc.vector.transpose(st[:], ps[:])
                    t = xpool.tile([128, 512], mmdt, tag="xt")
                    nc.scalar.copy(t[:], st[:])
                xt_next[h][mt] = t

            # ---- step 0. h=0 pass: k-outer so the PE consumes k-tiles in
            # DMA arrival order. h=1 pass: mt-outer -- its PSUM banks are
            # the recycled h=0 banks, each freed by that mt's DVE transpose;
            # mt-outer consumes them in exactly the order the serial
            # transpose chain releases them (k-outer would need all 8 banks
            # within the first k-slot and stall ~2 us).
            xt_next = [[None] * NK for _ in range(NHALF)]
            pss = [
                pspool.tile([128, 512], f32, tag="ps", name=f"ps0_0_{i}")
                for i in range(NM)
            ]
            for k in range(NK):
                for mt in range(NM):
                    nc.tensor.matmul(
                        pss[mt][:],
                        wsb[0][k][:, mt * 128 : (mt + 1) * 128],
                        x0[k][:, 0:512],
                        start=(k == 0),
                        stop=(k == NK - 1),
                    )
            for mt in range(NM):
                finish_tile(pss[mt], 0, mt, xt_next)
            for mt in range(NM):
                ps = pspool.tile([128, 512], f32, tag="ps", name=f"ps0_1_{mt}")
                for k in range(NK):
                    nc.tensor.matmul(
                        ps[:],
                        wsb[0][k][:, mt * 128 : (mt + 1) * 128],
                        x0[k][:, 512:1024],
                        start=(k == 0),
                        stop=(k == NK - 1),
                    )
                finish_tile(ps, 1, mt, xt_next)
            load_weights(2)
            xt = xt_next

            # ---- steps 1..4: mt-outer (staggers transposes across the step)
            for step in range(1, 5):
                xt_next = [[None] * NK for _ in range(NHALF)]
                for h in range(NHALF):
                    for mt in range(NM):
                        ps = pspool.tile([128, 512], f32, tag="ps")
                        for k in range(NK):
                            nc.tensor.matmul(
                                ps[:],
                                wsb[step][k][:, mt * 128 : (mt + 1) * 128],
                                xt[h][k][:],
                                start=(k == 0),
                                stop=(k == NK - 1),
                            )
                        finish_tile(ps, h, mt, xt_next)
                if step + 2 < NSTEP:
                    load_weights(step + 2)
                xt = xt_next

            # ---- step 5: state stationary, W moving; fused bias+relu+store ----
            from concourse.mybir import ActivationFunctionType

            # bias tile: [128, 1024], row p holds bias[(p%32)*1024 : ...];
            # loaded late, right before its only consumer.
            bias_sb = bpool.tile([128, 1024], f32, tag="bias")
            for r in range(4):
                nc.sync.dma_start(out=bias_sb[32 * r : 32 * (r + 1), :], in_=b2[:, :])

            for h in range(NHALF):
                for mc in range(4):  # output partition chunks of 128 (= 4 b values)
                    last_group = h == 1 and mc == 3
                    # the final group drains in N-quarters with stores
                    # alternating sync/scalar, so the post-last-matmul tail
                    # is one 64 KiB store instead of one 256 KiB store on a
                    # single ~78 GB/s queue.
                    nspl = 4 if last_group else 2
                    nw = 1024 // nspl
                    for nq in range(nspl):
                        ps = pspool.tile([128, nw], f32, tag="ps")
                        for k in range(NK):
                            nc.tensor.matmul(
                                ps[:],
                                xt[h][k][:, mc * 128 : (mc + 1) * 128],
                                wsb[5][k][:, nq * nw : (nq + 1) * nw],
                                start=(k == 0),
                                stop=(k == NK - 1),
                            )
                        tmp = tpool.tile([128, nw], f32, tag="tmp")
                        nc.vector.tensor_add(
                            tmp[:], ps[:], bias_sb[:, nq * nw : (nq + 1) * nw]
                        )
                        o = opool.tile([128, nw], f32, tag="o")
                        nc.scalar.activation(o[:], tmp[:], ActivationFunctionType.Relu)
                        b0 = h * HB + mc * 4
                        store_eng = nc.scalar if (last_group and nq % 2) else nc.sync
                        store_eng.dma_start(
                            out=y3[b0 : b0 + 4, :, nq * nw : (nq + 1) * nw],
                            in_=o[:],
                        )
    # Run the Bacc lowering passes (register allocation, wait splitting, ...)
    # — the PJRT execute path serializes nc.m as-is.
    nc.finalize()
    return nc


def _get_nc(mm_dtype_name):
    if mm_dtype_name not in _NC_CACHE:
        _NC_CACHE[mm_dtype_name] = _build_nc(mm_dtype_name)
    return _NC_CACHE[mm_dtype_name]


def _host_dt(mm_dtype_name):
    if mm_dtype_name == "float32r":
        return np.float32
    import ml_dtypes

    return np.dtype(getattr(ml_dtypes, mm_dtype_name))


def _prep_weights(nodes, host_dt):
    # W[i] layout [p=(u*32+v)%... rows 128 per k-tile packed as [128, 8*1024]]:
    # free index = k*1024 + col.  steps 0..4: col = n*32+m ; step 5: col = m*32+n.
    nodes = np.ascontiguousarray(nodes, dtype=np.float32)
    W = np.empty((NSTEP, 128, 8192), np.float32)
    for i in range(NSTEP):
        if i < 5:
            wm = nodes[i].reshape(1024, 32, 32).transpose(0, 2, 1).reshape(1024, 1024)
        else:
            wm = nodes[i].reshape(1024, 1024)
        # [k*128+p, col] -> [p, k*1024+col]
        W[i] = wm.reshape(NK, 128, 1024).transpose(1, 0, 2).reshape(128, 8192)
    return np.ascontiguousarray(W.astype(host_dt))


def _prep_x(x_core, host_dt):
    # [BC, 32768] -> [k, pp, b*32+f] with x[b, (k*128+pp)*32+f]
    return np.ascontiguousarray(
        x_core.reshape(BC, NK, 128, 32)
        .transpose(1, 2, 0, 3)
        .reshape(NK, 128, BC * 32)
        .astype(host_dt)
    )


def run(inputs, nodes, bias, mm_dtype="bfloat16", trace=False):
    from concourse.bass_utils import run_bass_kernel_spmd

    nc = _get_nc(mm_dtype)
    hdt = _host_dt(mm_dtype)
    x = np.ascontiguousarray(inputs, dtype=np.float32)
    bias = np.ascontiguousarray(bias, dtype=np.float32)
    W = _prep_weights(nodes, hdt)
    in_maps = [
        {"x": _prep_x(x[c * BC : (c + 1) * BC], hdt), "w": W, "bias_in": bias}
        for c in range(NCORES)
    ]
    res = run_bass_kernel_spmd(nc, in_maps, list(range(NCORES)), trace=trace)
    out = np.concatenate([res.results[c]["y"] for c in range(NCORES)], axis=0)
    return out, res


def kernel(inputs, nodes, bias):
    out, _ = run(inputs, nodes, bias)
    return out


# revision 19
# speedup vs baseline: 1.0340x; 1.0114x over previous
"""Trainium2 Bass kernel for nn_DenseEntangler (B=256, D=32, L=3, 6 nodes).

Math: out = relu(bias + chain of 6 tensordot contractions). Each per-sample
contraction is a (1024 x 1024) matmul applied to the reshaped state, so the
whole problem is 6 matmuls of [1024,1024]^T @ [1024, Bc*32] per core
(Bc = 32 samples/core on 8 cores, batch-sharded).

Layout scheme (verified against the reference in numpy):
  state XT[(u*32+v) partition, (b*32+f) free], K = 1024 -> 8 tiles of 128.
  steps 0..4:  OUT[(n*32+m), (b,f)] = W_i^T @ XT  with
               W_i[(u*32+v), (n*32+m)] = nodes[i][u,v,m,n]  (host pre-permute)
               transition to the next step's XT = independent aligned 32x32
               block transposes (swap partition-low m with free-low f) ->
               native DVE stream_transpose, runs off the PE critical path.
  step 5:      operands swapped (state stationary, W5 moving) so PSUM comes
               out as [(b*32+f) partition, (m*32+n) free], which is
               DRAM-contiguous per partition for the final store.

Perf design (measured on HW):
  - PE roofline: 6 steps x 128 matmuls x 512 rows @ 2.4 GHz ~= 164 us; the
    achievable cadence is 227 ns per [128x512] matmul and steps run
    back-to-back at exactly that once fed.
  - matmul dtype is bfloat16 (same 1 cycle/row PE rate as float32r at
    N>=256, but HALF the DMA bytes). The head needs x + w0 resident before
    step 0's h0 pass completes; at the ~260 GB/s the fabric sustains early,
    fp32 head data (8 MiB) stalls the PE for ~20 us while bf16 (3 MiB in
    deadline order) arrives ahead of consumption. PSUM accumulation stays
    fp32; per-step round-to-bf16 of the state adds ~0.5% RMS error total
    (gate is 2e-2).
  - x is pre-transposed on the host into the XT k-tile layout so the input
    DMA is contiguous (2 KiB rows) instead of 128-byte strided gathers.
  - head DMAs are emitted in k-deadline order interleaved across the
    gpsimd/sync/scalar queues; payload DMA cannot start before ~9 us
    (engine boot), so the first matmul lands ~13 us in regardless.
  - a short self-serializing fp32 warm-up matmul burst ramps the PE clock
    out of its 1.2 GHz p-state during the DMA dead window.
"""

import os
import sys

import numpy as np

for _p in ("/opt/trn_rl_repo", "/root/.axon_site/_ro/trn_rl_repo"):
    if _p not in sys.path and os.path.isdir(_p):
        sys.path.append(_p)

B = 256
NCORES = 8
BC = B // NCORES  # 32 samples per core
NSTEP = 6
NK = 8  # K tiles of 128 (K = 1024)
NM = 8  # output partition tiles of 128 (steps 0..4)
NHALF = 2  # halves of 16 samples -> moving free dim 512
HB = BC // NHALF  # 16
NWARM = 2  # PE clock warm-up matmuls (fp32, 4 cyc/row -> ~1.7us each cold)

_NC_CACHE = {}


def _build_nc(mm_dtype_name):
    import concourse.tile as tile
    from concourse import bacc, mybir

    f32 = mybir.dt.float32
    mmdt = getattr(mybir.dt, mm_dtype_name)
    is_f32r = mmdt == mybir.dt.float32r
    # float32r has no numpy representation: declare those params as f32 and
    # bitcast the DMA source; true 16-bit dtypes are declared directly and
    # cast on the host.
    decl_dt = f32 if is_f32r else mmdt

    def src(ap):
        return ap.bitcast(mmdt) if is_f32r else ap

    # Bacc (not plain Bass): its lowering runs move_matmul_waits_to_ldweights
    # + generate_event_semaphores, required to satisfy the HW 1-wait-per-
    # instruction constraint on fused LDWEIGHTS+MATMUL.
    nc = bacc.Bacc(None)
    # x arrives pre-transposed from the host: x[k, pp, b*32+f] =
    # inputs[b, (k*128+pp)*32 + f] -> every DMA row is contiguous.
    xh = nc.declare_dram_parameter("x", [NK, 128, BC * 32], decl_dt, isOutput=False)
    wh = nc.declare_dram_parameter("w", [NSTEP, 128, 8192], decl_dt, isOutput=False)
    bh = nc.declare_dram_parameter("bias_in", [32768], f32, isOutput=False)
    yh = nc.declare_dram_parameter("y", [BC, 32768], f32, isOutput=True)

    # bias[(f*1024 + q)] -> [f, q]
    b2 = bh[:].rearrange("(f q) -> f q", q=1024)
    # y[b, f*1024 + q] -> [b, f, q]
    y3 = yh[:, :].rearrange("b (f q) -> b f q", q=1024)

    with tile.TileContext(nc) as tc:
        with (
            tc.tile_pool(name="wpool", bufs=16) as wpool,
            tc.tile_pool(name="xpool", bufs=32) as xpool,
            tc.tile_pool(name="wupool", bufs=1) as wupool,
            tc.tile_pool(name="bpool", bufs=1) as bpool,
            tc.tile_pool(name="tpool", bufs=4) as tpool,
            tc.tile_pool(name="stpool", bufs=4) as stpool,
            tc.tile_pool(name="opool", bufs=4) as opool,
            tc.tile_pool(name="pspool", bufs=8, space="PSUM") as pspool,
        ):
            wsb = {}

            def load_weights(step, eng_of=None):
                # steady-state weight stream: even k -> gpsimd (SWDGE,
                # ~half the bytes), odd k split between the two HWDGE
                # queues. Well under per-queue limits at bf16 rates.
                if eng_of is None:
                    eng_of = lambda k: (
                        nc.gpsimd
                        if k % 2 == 0
                        else (nc.sync if k % 4 == 1 else nc.scalar)
                    )
                tiles = []
                for k in range(NK):
                    t = wpool.tile([128, 1024], mmdt, tag="w")
                    eng_of(k).dma_start(
                        out=t[:],
                        in_=src(wh[step, :, k * 1024 : (k + 1) * 1024]),
                    )
                    tiles.append(t)
                wsb[step] = tiles

            # ---- PE clock warm-up: self-serializing dummy matmuls on a
            # zeroed tile into a scratch PSUM bank. No data dependencies, so
            # they issue as soon as the engines boot and ramp the PE out of
            # its 1.2 GHz p-state while the first x/w tiles are in flight.
            wu = wupool.tile([128, 512], f32, tag="wu")
            nc.vector.memset(wu[:], 0)
            wups = pspool.tile([128, 512], f32, tag="ps", name="warmps")
            for _ in range(NWARM):
                nc.tensor.matmul(
                    wups[:], wu[:, 0:128], wu[:], start=True, stop=True
                )

            # ---- head. Measured per-queue caps: sync/scalar ~78-85 GB/s
            # each, gpsimd ~95-112 GB/s (byte-bound, not packet-bound), vs a
            # step-0 h0-pass demand of 384 KiB per 1.73 us k-slot (222 GB/s).
            # Every k-slot's bytes are split in equal thirds across the three
            # queues, in strict k order:
            #   gpsimd: x h0-half      (128K)
            #   sync:   w0 cols 0:512  (128K)
            #   scalar: w0 cols 512:.. (128K)
            # The x h1-halves follow on gpsimd (needed only by the h1 pass),
            # then w1 split sync/scalar, all ahead of their deadlines.
            x0 = [None] * NK
            wsb[0] = []
            for k in range(NK):
                x0[k] = xpool.tile(
                    [128, BC * 32], mmdt, tag="x0", name=f"x0_{k}", bufs=8
                )
                t = wpool.tile([128, 1024], mmdt, tag="w", name=f"w0_{k}")
                c0 = k * 1024
                nc.gpsimd.dma_start(
                    out=x0[k][:, 0 : HB * 32], in_=src(xh[k, :, 0 : HB * 32])
                )
                nc.sync.dma_start(
                    out=t[:, 0:512], in_=src(wh[0, :, c0 : c0 + 512])
                )
                nc.scalar.dma_start(
                    out=t[:, 512:1024], in_=src(wh[0, :, c0 + 512 : c0 + 1024])
                )
                wsb[0].append(t)
            for k in range(NK):
                nc.gpsimd.dma_start(
                    out=x0[k][:, HB * 32 : BC * 32],
                    in_=src(xh[k, :, HB * 32 : BC * 32]),
                )

            # w1: halves on the two HWDGE queues, done ~5 us before step 1.
            def load_weights_split(step):
                tiles = []
                for k in range(NK):
                    t = wpool.tile([128, 1024], mmdt, tag="w")
                    nc.sync.dma_start(
                        out=t[:, 0:512], in_=src(wh[step, :, k * 1024 : k * 1024 + 512])
                    )
                    nc.scalar.dma_start(
                        out=t[:, 512:1024],
                        in_=src(wh[step, :, k * 1024 + 512 : (k + 1) * 1024]),
                    )
                    tiles.append(t)
                wsb[step] = tiles

            load_weights_split(1)

            def finish_tile(ps, h, mt, xt_next):
                """PSUM -> (transpose, round-to-mmdt) -> next-step state tile."""
                if mmdt is f32:
                    t = xpool.tile([128, 512], f32, tag="xt")
                    nc.vector.transpose(t[:], ps[:])
                else:
                    st = stpool.tile([128, 512], f32, tag="st")
                    nc.vector.transpose(st[:], ps[:])
                    t = xpool.tile([128, 512], mmdt, tag="xt")
                    nc.scalar.copy(t[:], st[:])
                xt_next[h][mt] = t

            # ---- step 0. h=0 pass: k-outer so the PE consumes k-tiles in
            # DMA arrival order. h=1 pass: mt-outer -- its PSUM banks are
            # the recycled h=0 banks, each freed by that mt's DVE transpose;
            # mt-outer consumes them in exactly the order the serial
            # transpose chain releases them (k-outer would need all 8 banks
            # within the first k-slot and stall ~2 us).
            xt_next = [[None] * NK for _ in range(NHALF)]
            pss = [
                pspool.tile([128, 512], f32, tag="ps", name=f"ps0_0_{i}")
                for i in range(NM)
            ]
            for k in range(NK):
                for mt in range(NM):
                    nc.tensor.matmul(
                        pss[mt][:],
                        wsb[0][k][:, mt * 128 : (mt + 1) * 128],
                        x0[k][:, 0:512],
                        start=(k == 0),
                        stop=(k == NK - 1),
                    )
            for mt in range(NM):
                finish_tile(pss[mt], 0, mt, xt_next)
            for mt in range(NM):
                ps = pspool.tile([128, 512], f32, tag="ps", name=f"ps0_1_{mt}")
                for k in range(NK):
                    nc.tensor.matmul(
                        ps[:],
                        wsb[0][k][:, mt * 128 : (mt + 1) * 128],
                        x0[k][:, 512:1024],
                        start=(k == 0),
                        stop=(k == NK - 1),
                    )
                finish_tile(ps, 1, mt, xt_next)
            load_weights(2)
            xt = xt_next

            # ---- steps 1..4: mt-outer (staggers transposes across the step)
            for step in range(1, 5):
                xt_next = [[None] * NK for _ in range(NHALF)]
                for h in range(NHALF):
                    for mt in range(NM):
                        ps = pspool.tile([128, 512], f32, tag="ps")
                        for k in range(NK):
                            nc.tensor.matmul(
                                ps[:],
                                wsb[step][k][:, mt * 128 : (mt + 1) * 128],
                                xt[h][k][:],
                                start=(k == 0),
                                stop=(k == NK - 1),
                            )
                        finish_tile(ps, h, mt, xt_next)
                if step + 2 < NSTEP:
                    load_weights(step + 2)
                xt = xt_next

            # ---- step 5: state stationary, W moving; fused bias+relu+store ----
            from concourse.mybir import ActivationFunctionType

            # bias tile: [128, 1024], row p holds bias[(p%32)*1024 : ...];
            # loaded late, right before its only consumer.
            bias_sb = bpool.tile([128, 1024], f32, tag="bias")
            for r in range(4):
                nc.sync.dma_start(out=bias_sb[32 * r : 32 * (r + 1), :], in_=b2[:, :])

            # stores rotate across all three DGE queues: 4 MiB of output on
            # one ~78 GB/s queue (51 us) cannot drain inside step 5's ~28 us
            # and backlogs the final chunks by ~4 us.
            store_qs = [nc.sync, nc.scalar, nc.gpsimd]
            store_i = 0
            for h in range(NHALF):
                for mc in range(4):  # output partition chunks of 128 (= 4 b values)
                    last_group = h == 1 and mc == 3
                    # the final group drains in N-quarters so the
                    # post-last-matmul tail is one 64 KiB store.
                    nspl = 4 if last_group else 2
                    nw = 1024 // nspl
                    for nq in range(nspl):
                        ps = pspool.tile([128, nw], f32, tag="ps")
                        for k in range(NK):
                            nc.tensor.matmul(
                                ps[:],
                                xt[h][k][:, mc * 128 : (mc + 1) * 128],
                                wsb[5][k][:, nq * nw : (nq + 1) * nw],
                                start=(k == 0),
                                stop=(k == NK - 1),
                            )
                        tmp = tpool.tile([128, nw], f32, tag="tmp")
                        nc.vector.tensor_add(
                            tmp[:], ps[:], bias_sb[:, nq * nw : (nq + 1) * nw]
                        )
                        o = opool.tile([128, nw], f32, tag="o")
                        nc.scalar.activation(o[:], tmp[:], ActivationFunctionType.Relu)
                        b0 = h * HB + mc * 4
                        store_qs[store_i % 3].dma_start(
                            out=y3[b0 : b0 + 4, :, nq * nw : (nq + 1) * nw],
                            in_=o[:],
                        )
                        store_i += 1
    # Run the Bacc lowering passes (register allocation, wait splitting, ...)
    # — the PJRT execute path serializes nc.m as-is.
    nc.finalize()
    return nc


def _get_nc(mm_dtype_name):
    if mm_dtype_name not in _NC_CACHE:
        _NC_CACHE[mm_dtype_name] = _build_nc(mm_dtype_name)
    return _NC_CACHE[mm_dtype_name]


def _host_dt(mm_dtype_name):
    if mm_dtype_name == "float32r":
        return np.float32
    import ml_dtypes

    return np.dtype(getattr(ml_dtypes, mm_dtype_name))


def _prep_weights(nodes, host_dt):
    # W[i] layout [p=(u*32+v)%... rows 128 per k-tile packed as [128, 8*1024]]:
    # free index = k*1024 + col.  steps 0..4: col = n*32+m ; step 5: col = m*32+n.
    nodes = np.ascontiguousarray(nodes, dtype=np.float32)
    W = np.empty((NSTEP, 128, 8192), np.float32)
    for i in range(NSTEP):
        if i < 5:
            wm = nodes[i].reshape(1024, 32, 32).transpose(0, 2, 1).reshape(1024, 1024)
        else:
            wm = nodes[i].reshape(1024, 1024)
        # [k*128+p, col] -> [p, k*1024+col]
        W[i] = wm.reshape(NK, 128, 1024).transpose(1, 0, 2).reshape(128, 8192)
    return np.ascontiguousarray(W.astype(host_dt))


def _prep_x(x_core, host_dt):
    # [BC, 32768] -> [k, pp, b*32+f] with x[b, (k*128+pp)*32+f]
    return np.ascontiguousarray(
        x_core.reshape(BC, NK, 128, 32)
        .transpose(1, 2, 0, 3)
        .reshape(NK, 128, BC * 32)
        .astype(host_dt)
    )


def run(inputs, nodes, bias, mm_dtype="bfloat16", trace=False):
    from concourse.bass_utils import run_bass_kernel_spmd

    nc = _get_nc(mm_dtype)
    hdt = _host_dt(mm_dtype)
    x = np.ascontiguousarray(inputs, dtype=np.float32)
    bias = np.ascontiguousarray(bias, dtype=np.float32)
    W = _prep_weights(nodes, hdt)
    in_maps = [
        {"x": _prep_x(x[c * BC : (c + 1) * BC], hdt), "w": W, "bias_in": bias}
        for c in range(NCORES)
    ]
    res = run_bass_kernel_spmd(nc, in_maps, list(range(NCORES)), trace=trace)
    out = np.concatenate([res.results[c]["y"] for c in range(NCORES)], axis=0)
    return out, res


def kernel(inputs, nodes, bias):
    out, _ = run(inputs, nodes, bias)
    return out
